# revision 4
# baseline (speedup 1.0000x reference)
"""Multi-head attention (B=2, S=2048, E=1024, H=16) on 8 Trainium2 NeuronCores.

Sharding: data-parallel over the 2 batches x tensor-parallel over 4 head-groups
(4 heads each).  Core c handles batch c//4, heads [4*(c%4), 4*(c%4)+4).
Each core computes its heads' Q/K/V projections, softmax(QK^T/8)V, and the
partial output projection against the matching Wo column slice; the host sums
the 4 partials per batch (the head-parallel all-reduce) and stacks batches.

Device-side layout notes:
 - Host pre-transposes x and the weight slices so every matmul operand already
   has its contraction dim on SBUF partitions (no on-device transposes).
 - The whole PE stream is 16-bit: x/W/q/k/v/Wo/concat are bf16 and the exp
   output is fp16.  PSUM accumulation is fp32 throughout, and y partials are
   bf16 summed in f32 on the host.
 - Scores are produced transposed, sT[j, i] = k_j . q_i, so softmax(j) is a
   partition-dim reduction folded into the P@V matmul via a ones column on V
   (out row 64 = sum_j exp(sT[j, i])), and the attention output lands directly
   in the [head_dim, seq] layout the output projection needs as lhsT.
 - exp runs on the scalar engine straight out of PSUM with the 1/sqrt(dk)
   scale and a constant -4 bias folded in (softmax is shift-invariant).
 - The kernel is scalar(exp)-bound in steady state (128 exps of [128,1024]
   ~= 142us back-to-back); everything else is scheduled to keep that stream
   gap-free:
    * x chunk 0 is DMA'd as four ec-pair sub-tiles so the first K/Q
      projection chains pipeline per-ec behind the DMA and the first exp
      starts as early as possible; the QT(0,0) drain goes through the (still
      idle) scalar engine so the K/Q drains run in parallel.
    * the attention loop runs per j-tile with filler units (projections,
      output tiles) popped between each exp and the previous tile's PVs.
    * softmax normalization runs off the PE: DVE fast-reciprocal + gpsimd
      partition_broadcast + DVE multiply, so head-pair/chunk boundaries never
      stall the PE on the DVE chain, and the next chunk's scores issue
      immediately after the last PV.
    * junk warm matmuls only pad the input-DMA window (HAM clock warmup);
      the tail runs on real work only.
"""

import heapq

import numpy as np
import ml_dtypes

import concourse.bass as bass
from concourse import bacc
import concourse.mybir as mybir
import concourse.tile as tile
from concourse.bass_utils import run_bass_kernel_spmd

B, S, E, H = 2, 2048, 1024, 16
DK = 64
NCORES = 8
HGROUPS = 4            # head-parallel groups per batch
HLOC = H // HGROUPS    # heads per core = 4
FH = HLOC * DK         # local feature cols = 256

F32 = mybir.dt.float32
BF16 = mybir.dt.bfloat16
EXP_BIAS = -4.0        # constant shift inside exp; cancels in softmax


def _build_program() -> bass.Bass:
    nc = bacc.Bacc("TRN2", target_bir_lowering=False, debug=False,
                   enable_asserts=False)

    # inputs are pre-arranged on the host into the exact SBUF tile layout so
    # every input DMA is fully contiguous per partition line.
    xt_d = nc.dram_tensor("xt", [4, 128, E // 128, 512], BF16,
                          kind="ExternalInput").ap()
    wqt_d = nc.dram_tensor("wqt", [128, E // 128, FH], BF16,
                           kind="ExternalInput").ap()
    wkt_d = nc.dram_tensor("wkt", [128, E // 128, FH], BF16,
                           kind="ExternalInput").ap()
    wvt_d = nc.dram_tensor("wvt", [128, E // 128, FH], BF16,
                           kind="ExternalInput").ap()
    wot_d = nc.dram_tensor("wot", [128, FH // 128, E], BF16,
                           kind="ExternalInput").ap()
    y_d = nc.dram_tensor("y", [S, E], BF16, kind="ExternalOutput").ap()

    EC = E // 128        # 8 contraction chunks for the projections
    ST = S // 128        # 16 seq tiles of 128 (the j tiles)
    SC = S // 512        # 4 seq chunks of 512 (the i chunks)
    FT = FH // 128       # 2 feature tiles (head pairs)

    with tile.TileContext(nc) as tc:
        with (
            tc.tile_pool(name="constp", bufs=1) as constp,
            tc.tile_pool(name="xtp", bufs=SC + 3) as xtp,
            tc.tile_pool(name="wp", bufs=1) as wp,
            tc.tile_pool(name="qkp", bufs=2 * FT * SC) as qkp,
            tc.tile_pool(name="vp", bufs=ST) as vp,
            tc.tile_pool(name="cp", bufs=3) as cp,
            tc.tile_pool(name="ep", bufs=8) as ep,
            tc.tile_pool(name="aup", bufs=8) as aup,
            tc.tile_pool(name="smp", bufs=10) as smp,
            tc.tile_pool(name="bcp", bufs=4) as bcp,
            tc.tile_pool(name="op", bufs=6) as op,
            tc.tile_pool(name="mmp", bufs=2, space="PSUM") as mmp,
            tc.tile_pool(name="scp", bufs=2, space="PSUM") as scp,
            tc.tile_pool(name="atp", bufs=2, space="PSUM") as atp,
        ):
            ones = constp.tile([128, DK], BF16, tag="ones")
            nc.vector.memset(ones[:], 1.0)
            bias_t = constp.tile([128, 1], F32, tag="bias")
            nc.vector.memset(bias_t[:], EXP_BIAS)
            onescol = constp.tile([128, HLOC], BF16, tag="onescol")
            nc.vector.memset(onescol[:], 1.0)
            warm = constp.tile([128, 512], BF16, tag="warm")
            nc.vector.memset(warm[:], 1.0)

            # ---- input DMAs ----
            # Order matters: the first-exp critical path is WK, WQ, x chunk 0.
            # x0 arrives as four ec-pair sub-tiles so the first K/Q chains can
            # pipeline per-ec behind the DMA instead of waiting for the whole
            # 1MB transfer.
            WQ = wp.tile([128, EC, FH], BF16, tag="wq")
            WK = wp.tile([128, EC, FH], BF16, tag="wk")
            WV = wp.tile([128, EC, FH], BF16, tag="wv")
            WO = wp.tile([128, FT, E], BF16, tag="wo")
            nc.sync.dma_start(WK[:], wkt_d)
            nc.sync.dma_start(WQ[:], wqt_d)
            X0 = []
            for g in range(4):
                x0g = xtp.tile([128, 2, 512], BF16, tag="xt", name=f"x0_{g}")
                nc.sync.dma_start(x0g[:], xt_d[0][:, 2 * g:2 * g + 2, :])
                X0.append(x0g)
            nc.sync.dma_start(WV[:], wvt_d)
            XSC = [None]
            for sc in range(1, SC):
                xtile = xtp.tile([128, EC, 512], BF16, tag="xt",
                                 name=f"xt_{sc}")
                nc.sync.dma_start(xtile[:], xt_d[sc])
                XSC.append(xtile)
            # Wo is first used ~40 iterations in; keep it behind all of x.
            nc.sync.dma_start(WO[:], wot_d)

            def xap(sc, ec):
                """x operand for contraction chunk ec of seq chunk sc."""
                if sc == 0:
                    return X0[ec // 2][:, ec % 2, :]
                return XSC[sc][:, ec, :]

            # ---- PE warmup during the input-DMA window ----
            # The HAM clock gate starts at half clock and needs ~3.4us of
            # sustained PE activity; burn cheap bf16 matmuls on junk data
            # while the inputs stream in so the first chains run warm.  They
            # write a scores-pool bank that is untouched until the first
            # scores, so they wait on nothing.  ~22 at the cold/warm mix ends
            # right as the first x sub-tiles land, so the first K/Q chains
            # pipeline behind the DMA instead of queueing behind junk.
            ps_w = scp.tile([128, 1024], F32, tag="sc", name="warmps")
            for _ in range(22):
                nc.tensor.matmul(ps_w[:, 0:512], warm[:, 0:128], warm[:, :],
                                 start=True, stop=True)

            # ---- filler unit scheduler ----
            # Each unit is a small PE/DVE/gpsimd burst; run_at is the
            # attention iteration (ic*32 + ft*16 + jt) it should run right
            # before.  run_at also fixes instruction emission order, so it
            # must respect dataflow.
            pending = []
            seq = [0]

            def add(run_at, fn):
                heapq.heappush(pending, (run_at, seq[0], fn))
                seq[0] += 1

            def pop_units(it):
                while pending and pending[0][0] <= it:
                    heapq.heappop(pending)[2]()

            # ---- projections ----
            # qT/kT: [f, s] layout.  out[f_tile, s_chunk] = sum_ec WqT^T @ xT
            QTs = {}
            KTs = {}

            def qk_proj_half(store, w, ft, sc, half, state):
                """One half (4 ec chunks) of a q/k projection chain."""
                if half == 0:
                    state["ps"] = mmp.tile([128, 512], F32, tag="mm",
                                           name="qkps")
                ps = state["ps"]
                for ec in range(4 * half, 4 * half + 4):
                    nc.tensor.matmul(
                        ps[:, :],
                        w[:, ec, ft * 128:(ft + 1) * 128],
                        xap(sc, ec),
                        start=(ec == 0), stop=(ec == EC - 1),
                    )
                if half == 1:
                    dst = qkp.tile([128, 512], BF16, tag="qk",
                                   name=f"qk_{ft}_{sc}_{len(store)}")
                    nc.vector.tensor_copy(dst[:], ps[:, :])
                    store[(ft, sc)] = dst

            def sched_qk(store, w, ft, sc, run_at):
                st = {}
                add(run_at - 1,
                    lambda: qk_proj_half(store, w, ft, sc, 0, st))
                add(run_at,
                    lambda: qk_proj_half(store, w, ft, sc, 1, st))

            # v: natural [s, f] layout, plus a fused ones column per head:
            # VAUG[jt] is [128, HLOC, DK+1] with [:, h, DK] == 1.
            VAUG = [None] * ST

            def v_proj(jt):
                va = vp.tile([128, HLOC, DK + 1], BF16, tag="vaug")
                nc.vector.tensor_copy(va[:, :, DK:DK + 1],
                                      onescol[:, :, None])
                ps = mmp.tile([128, 512], F32, tag="mm", name="vps")
                sc = jt // 4
                for ec in range(EC):
                    nc.tensor.matmul(
                        ps[:, 0:FH],
                        xap(sc, ec)[:, (jt % 4) * 128:(jt % 4 + 1) * 128],
                        WV[:, ec, :],
                        start=(ec == 0), stop=(ec == EC - 1),
                    )
                nc.vector.tensor_copy(
                    va[:, :, 0:DK],
                    ps[:, 0:FH].rearrange("p (h d) -> p h d", d=DK))
                VAUG[jt] = va

            # ---- prologue: first K/Q chains, interleaved per-ec so both
            # pipeline behind the x0 sub-DMAs.  The Q drain goes through the
            # scalar engine (idle until the first exp) so the two drains
            # overlap instead of queueing on the DVE.
            ps_k = mmp.tile([128, 512], F32, tag="mm", name="kps0")
            ps_q = mmp.tile([128, 512], F32, tag="mm", name="qps0")
            for ec in range(EC):
                nc.tensor.matmul(ps_k[:, :], WK[:, ec, 0:128], xap(0, ec),
                                 start=(ec == 0), stop=(ec == EC - 1))
                nc.tensor.matmul(ps_q[:, :], WQ[:, ec, 0:128], xap(0, ec),
                                 start=(ec == 0), stop=(ec == EC - 1))
            kt00 = qkp.tile([128, 512], BF16, tag="qk", name="kt00")
            nc.vector.tensor_copy(kt00[:], ps_k[:, :])
            KTs[(0, 0)] = kt00
            qt00 = qkp.tile([128, 512], BF16, tag="qk", name="qt00")
            nc.scalar.activation(qt00[:], ps_q[:, :],
                                 mybir.ActivationFunctionType.Copy)
            QTs[(0, 0)] = qt00

            # ---- filler schedule ----
            # V tiles just-in-time: VAUG[jt] is consumed by PV(jt), emitted at
            # iteration jt+1; popping at jt keeps each chain in its own slot.
            for _jt in range(ST):
                add(_jt, lambda jt=_jt: v_proj(jt))
            for _sc in range(1, SC):             # KTs[(0,sc)] used from 4*sc
                sched_qk(KTs, WK, 0, _sc, 4 * _sc - 2)
            sched_qk(QTs, WQ, 1, 0, 11)          # used from iter 16
            for _sc in range(SC):                # KTs[(1,sc)] used from 16+4sc
                sched_qk(KTs, WK, 1, _sc, 13 + 4 * _sc)
            for _ic in range(1, SC):
                # QTs[(0,ic)] is used from 32*ic; QTs[(1,ic)] from 32*ic+16.
                sched_qk(QTs, WQ, 0, _ic, 32 * _ic - 16)
                sched_qk(QTs, WQ, 1, _ic, 32 * _ic)

            def phase_c(ic, concat, start_at):
                # output projection: 8 tiles of [128 s, 512 e] per i chunk,
                # spread two iterations apart through the next chunk's loop.
                k = 0
                for stl in range(4):
                    st = ic * 4 + stl
                    for oc in range(2):
                        def emit(st=st, oc=oc, stl=stl, concat=concat):
                            ps_o = mmp.tile([128, 512], F32, tag="mm",
                                            name="ops")
                            for fc in range(FT):
                                nc.tensor.matmul(
                                    ps_o[:, :],
                                    concat[:, fc, stl * 128:(stl + 1) * 128],
                                    WO[:, fc, oc * 512:(oc + 1) * 512],
                                    start=(fc == 0), stop=(fc == FT - 1),
                                )
                            ot = op.tile([128, 512], BF16, tag="out")
                            if ic == SC - 1 and (stl + oc) % 2 == 0:
                                # tail tiles: the scalar engine is idle after
                                # the last exp, use it for half the drains
                                nc.scalar.activation(
                                    ot[:], ps_o[:, :],
                                    mybir.ActivationFunctionType.Copy)
                            else:
                                nc.vector.tensor_copy(ot[:], ps_o[:, :])
                            nc.sync.dma_start(
                                y_d[st * 128:(st + 1) * 128,
                                    oc * 512:(oc + 1) * 512],
                                ot[:])
                        add(start_at + 2 * k, emit)
                        k += 1

            def normalize_half(concat, aus, dnh, ft, start_at):
                # fast approx reciprocal (f32, ~18 bits) of this head pair's
                # denominators (rows 0/32 of dnh), cast to bf16 for the
                # broadcast matmuls, then two broadcast-matmul + multiply
                # pairs writing this pair's concat half.
                rdf = smp.tile([33, 512], F32, tag="rdf")
                rd = smp.tile([33, 512], BF16, tag="rd")

                def recip(rdf=rdf, rd=rd, dnh=dnh):
                    nc.vector.reciprocal_approx_fast(rdf[:], dnh[:])
                    nc.vector.tensor_copy(rd[:], rdf[:])
                add(start_at, recip)

                for hs in range(2):
                    # both heads' units at the same iteration: the broadcast
                    # matmuls then sit adjacent in the PE queue and overlap
                    # as concurrent 32-row groups
                    def norm_h(hs=hs, rd=rd, concat=concat, ft=ft,
                               aus=tuple(aus)):
                        pb = hs * DK
                        ps_b = mmp.tile([DK, 512], F32, tag="mm", name="bc")
                        nc.tensor.matmul(ps_b[:, :],
                                         ones[hs * 32:hs * 32 + 1, :],
                                         rd[hs * 32:hs * 32 + 1, :],
                                         start=True, stop=True,
                                         tile_position=(hs * 32, 0))
                        nc.vector.tensor_tensor(
                            concat[pb:pb + DK, ft, :], aus[hs][:, :],
                            ps_b[:, :], mybir.AluOpType.mult)
                    add(start_at + 1, norm_h)

            # ---- attention + output projection, per 512-wide i chunk ----
            # Heads are processed in pairs (partition bases 0/64); the K=64
            # score matmuls co-issue in distinct PE row groups.  The loop is
            # per j-tile: scores(jt), exp(jt), filler pop, then PV(jt-1), so
            # the exp stream stays back-to-back while the PE fills its slack
            # with projection/output units.
            for ic in range(SC):
                concat = cp.tile([128, FT, 512], BF16, tag="concat")
                base = ic * 32
                for ft in range(FT):
                    ps_h = [atp.tile([128, 512], F32, tag="at",
                                     name=f"at_{ic}_{ft}_{hs}")
                            for hs in range(2)]
                    prev_pv = None
                    for jt in range(ST):
                        ps_s = scp.tile([128, 1024], F32, tag="sc")
                        for hs in range(2):
                            pb = hs * DK
                            nc.tensor.matmul(
                                ps_s[:, hs * 512:(hs + 1) * 512],
                                KTs[(ft, jt // 4)][pb:pb + DK,
                                                   (jt % 4) * 128:
                                                   (jt % 4 + 1) * 128],
                                QTs[(ft, ic)][pb:pb + DK, :],
                                start=True, stop=True,
                            )
                        ex = ep.tile([128, 1024], mybir.dt.float16,
                                     tag="exp")
                        nc.scalar.activation(
                            ex[:], ps_s[:],
                            mybir.ActivationFunctionType.Exp,
                            bias=bias_t[:], scale=1.0 / np.sqrt(DK))
                        pop_units(base + ft * 16 + jt)
                        if prev_pv is not None:
                            prev_pv()

                        def pv(jt=jt, ft=ft, ps_h=ps_h, ex=ex):
                            for hs in range(2):
                                nc.tensor.matmul(
                                    ps_h[hs][0:DK + 1, :],
                                    VAUG[jt][:, ft * 2 + hs, :],
                                    ex[:, hs * 512:(hs + 1) * 512],
                                    start=(jt == 0), stop=(jt == ST - 1),
                                )
                        prev_pv = pv
                    prev_pv()
                    # free the attention psum quickly: attn rows via DVE
                    # copies; at the very end of the kernel (no exp follows)
                    # one attn copy and one denominator row go through the
                    # idle scalar engine as well.
                    last = (ic == SC - 1 and ft == FT - 1)
                    aus = []
                    dnh = smp.tile([33, 512], F32, tag="dn")
                    for hs in range(2):
                        au = aup.tile([DK, 512], BF16, tag="au")
                        if hs == 0 and last:
                            nc.scalar.activation(
                                au[:], ps_h[0][0:DK, :],
                                mybir.ActivationFunctionType.Copy)
                        else:
                            nc.vector.tensor_copy(au[:], ps_h[hs][0:DK, :])
                        aus.append(au)
                        if hs == 0 and last:
                            nc.scalar.activation(
                                dnh[0:1, :], ps_h[0][DK:DK + 1, :],
                                mybir.ActivationFunctionType.Copy)
                        else:
                            nc.vector.tensor_copy(
                                dnh[hs * 32:hs * 32 + 1, :],
                                ps_h[hs][DK:DK + 1, :])
                    normalize_half(concat, aus, dnh, ft,
                                   base + ft * 16 + 17)
                phase_c(ic, concat, base + 35)

            # Tail flush: the last chunk's normalize + output tiles, real
            # work only.  The PE gaps here (waiting on DVE/gpsimd normalize)
            # are well under the ~3.4us HAM window, so the clock stays warm
            # without junk padding.
            for _, _, fn in sorted(pending):
                fn()

    nc.compile()
    return nc


_PROGRAM = None


def _get_program() -> bass.Bass:
    global _PROGRAM
    if _PROGRAM is None:
        _PROGRAM = _build_program()
    return _PROGRAM


def _prepare_in_maps(x, Wq, Wk, Wv, Wo):
    x = np.asarray(x, dtype=np.float32)
    Wq = np.asarray(Wq, dtype=np.float32)
    Wk = np.asarray(Wk, dtype=np.float32)
    Wv = np.asarray(Wv, dtype=np.float32)
    Wo = np.asarray(Wo, dtype=np.float32)
    bf = ml_dtypes.bfloat16
    in_maps = []
    for c in range(NCORES):
        b, hg = c // HGROUPS, c % HGROUPS
        rows = slice(hg * FH, (hg + 1) * FH)
        # device tile layouts: x -> [sc][p, c, s], W -> [p, c, f]
        # where the contraction index e = c*128 + p
        xt = x[b].T.reshape(E // 128, 128, S)
        xt = np.stack([xt[:, :, sc * 512:(sc + 1) * 512].transpose(1, 0, 2)
                       for sc in range(4)])
        wq = Wq[rows, :].T.reshape(E // 128, 128, FH).transpose(1, 0, 2)
        wk = Wk[rows, :].T.reshape(E // 128, 128, FH).transpose(1, 0, 2)
        wv = Wv[rows, :].T.reshape(E // 128, 128, FH).transpose(1, 0, 2)
        wo = Wo[:, rows].T.reshape(FH // 128, 128, E).transpose(1, 0, 2)
        in_maps.append({
            "xt": np.ascontiguousarray(xt).astype(bf),
            "wqt": np.ascontiguousarray(wq).astype(bf),
            "wkt": np.ascontiguousarray(wk).astype(bf),
            "wvt": np.ascontiguousarray(wv).astype(bf),
            "wot": np.ascontiguousarray(wo).astype(bf),
        })
    return in_maps


def run(inputs: dict, **spmd_kwargs):
    """Run on all 8 cores; returns (full output, BassKernelResults)."""
    nc = _get_program()
    in_maps = _prepare_in_maps(**inputs)
    res = run_bass_kernel_spmd(nc, in_maps, core_ids=list(range(NCORES)),
                               **spmd_kwargs)
    partials = [r["y"] for r in res.results]
    out = np.empty((B, S, E), dtype=np.float32)
    for b in range(B):
        acc = partials[b * HGROUPS].astype(np.float32, copy=True)
        for hg in range(1, HGROUPS):
            acc += partials[b * HGROUPS + hg]
        out[b] = acc
    return out, res


def kernel(**inputs) -> np.ndarray:
    out, _ = run(inputs)
    return out


# revision 12
# speedup vs baseline: 1.0049x; 1.0049x over previous
"""Multi-head attention (B=2, S=2048, E=1024, H=16) on 8 Trainium2 NeuronCores.

Sharding: data-parallel over the 2 batches x tensor-parallel over 4 head-groups
(4 heads each).  Core c handles batch c//4, heads [4*(c%4), 4*(c%4)+4).
Each core computes its heads' Q/K/V projections, softmax(QK^T/8)V, and the
partial output projection against the matching Wo column slice; the host sums
the 4 partials per batch (the head-parallel all-reduce) and stacks batches.

Device-side layout notes:
 - Host pre-transposes x and the weight slices so every matmul operand already
   has its contraction dim on SBUF partitions (no on-device transposes).
 - The whole PE stream is 16-bit: x/W/q/k/v/Wo/concat are bf16 and the exp
   output is fp16.  PSUM accumulation is fp32 throughout, and y partials are
   bf16 summed in f32 on the host.
 - Scores are produced transposed, sT[j, i] = k_j . q_i, so softmax(j) is a
   partition-dim reduction folded into the P@V matmul via a ones column on V
   (out row 64 = sum_j exp(sT[j, i])), and the attention output lands directly
   in the [head_dim, seq] layout the output projection needs as lhsT.
 - exp runs on the scalar engine straight out of PSUM with the 1/sqrt(dk)
   scale and a constant -4 bias folded in (softmax is shift-invariant).
 - The kernel is scalar(exp)-bound in steady state (128 exps of [128,1024]
   ~= 142us back-to-back); everything else is scheduled to keep that stream
   gap-free:
    * x chunk 0 is DMA'd as four ec-pair sub-tiles so the first K/Q
      projection chains pipeline per-ec behind the DMA and the first exp
      starts as early as possible; the QT(0,0) drain goes through the (still
      idle) scalar engine so the K/Q drains run in parallel.
    * the attention loop runs per j-tile with filler units (projections,
      output tiles) popped between each exp and the previous tile's PVs.
    * softmax normalization runs off the PE: DVE fast-reciprocal + gpsimd
      partition_broadcast + DVE multiply, so head-pair/chunk boundaries never
      stall the PE on the DVE chain, and the next chunk's scores issue
      immediately after the last PV.
    * junk warm matmuls only pad the input-DMA window (HAM clock warmup);
      the tail runs on real work only.
"""

import heapq

import numpy as np
import ml_dtypes

import concourse.bass as bass
from concourse import bacc
import concourse.mybir as mybir
import concourse.tile as tile
from concourse.bass_utils import run_bass_kernel_spmd

B, S, E, H = 2, 2048, 1024, 16
DK = 64
NCORES = 8
HGROUPS = 4            # head-parallel groups per batch
HLOC = H // HGROUPS    # heads per core = 4
FH = HLOC * DK         # local feature cols = 256

F32 = mybir.dt.float32
BF16 = mybir.dt.bfloat16
EXP_BIAS = -4.0        # constant shift inside exp; cancels in softmax


def _build_program() -> bass.Bass:
    nc = bacc.Bacc("TRN2", target_bir_lowering=False, debug=False,
                   enable_asserts=False)

    # inputs are pre-arranged on the host into the exact SBUF tile layout so
    # every input DMA is fully contiguous per partition line.
    xt_d = nc.dram_tensor("xt", [4, 128, E // 128, 512], BF16,
                          kind="ExternalInput").ap()
    wqt_d = nc.dram_tensor("wqt", [128, E // 128, FH], BF16,
                           kind="ExternalInput").ap()
    wkt_d = nc.dram_tensor("wkt", [128, E // 128, FH], BF16,
                           kind="ExternalInput").ap()
    wvt_d = nc.dram_tensor("wvt", [128, E // 128, FH], BF16,
                           kind="ExternalInput").ap()
    wot_d = nc.dram_tensor("wot", [128, FH // 128, E], BF16,
                           kind="ExternalInput").ap()
    y_d = nc.dram_tensor("y", [S, E], BF16, kind="ExternalOutput").ap()

    EC = E // 128        # 8 contraction chunks for the projections
    ST = S // 128        # 16 seq tiles of 128 (the j tiles)
    SC = S // 512        # 4 seq chunks of 512 (the i chunks)
    FT = FH // 128       # 2 feature tiles (head pairs)

    with tile.TileContext(nc) as tc:
        with (
            tc.tile_pool(name="constp", bufs=1) as constp,
            tc.tile_pool(name="xtp", bufs=SC + 3) as xtp,
            tc.tile_pool(name="wp", bufs=1) as wp,
            tc.tile_pool(name="qkp", bufs=2 * FT * SC) as qkp,
            tc.tile_pool(name="vp", bufs=ST) as vp,
            tc.tile_pool(name="cp", bufs=3) as cp,
            tc.tile_pool(name="ep", bufs=10) as ep,
            tc.tile_pool(name="aup", bufs=8) as aup,
            tc.tile_pool(name="smp", bufs=10) as smp,
            tc.tile_pool(name="bcp", bufs=4) as bcp,
            tc.tile_pool(name="op", bufs=6) as op,
            tc.tile_pool(name="mmp", bufs=2, space="PSUM") as mmp,
            tc.tile_pool(name="scp", bufs=2, space="PSUM") as scp,
            tc.tile_pool(name="atp", bufs=2, space="PSUM") as atp,
        ):
            ones = constp.tile([128, DK], BF16, tag="ones")
            nc.vector.memset(ones[:], 1.0)
            bias_t = constp.tile([128, 1], F32, tag="bias")
            nc.vector.memset(bias_t[:], EXP_BIAS)
            onescol = constp.tile([128, HLOC], BF16, tag="onescol")
            nc.vector.memset(onescol[:], 1.0)
            warm = constp.tile([128, 512], BF16, tag="warm")
            nc.vector.memset(warm[:], 1.0)

            # ---- input DMAs ----
            # Order matters: the first-exp critical path is WK, WQ, x chunk 0.
            # WK/WQ arrive as two ec-half tiles and x0 as four ec-pair
            # sub-tiles, interleaved so the first K/Q chains pipeline per-ec
            # behind the DMA instead of waiting for whole transfers.
            WKH = [wp.tile([128, 4, FH], BF16, tag=f"wk{h}", name=f"wk_{h}")
                   for h in range(2)]
            WQH = [wp.tile([128, 4, FH], BF16, tag=f"wq{h}", name=f"wq_{h}")
                   for h in range(2)]
            WV = wp.tile([128, EC, FH], BF16, tag="wv")
            WO = wp.tile([128, FT, E], BF16, tag="wo")
            X0 = []

            def x0_dma(g):
                x0g = xtp.tile([128, 2, 512], BF16, tag="xt", name=f"x0_{g}")
                nc.sync.dma_start(x0g[:], xt_d[0][:, 2 * g:2 * g + 2, :])
                X0.append(x0g)

            nc.sync.dma_start(WKH[0][:], wkt_d[:, 0:4, :])
            nc.sync.dma_start(WQH[0][:], wqt_d[:, 0:4, :])
            x0_dma(0)
            x0_dma(1)
            nc.sync.dma_start(WKH[1][:], wkt_d[:, 4:8, :])
            nc.sync.dma_start(WQH[1][:], wqt_d[:, 4:8, :])
            x0_dma(2)
            x0_dma(3)
            nc.sync.dma_start(WV[:], wvt_d)
            XSC = [None]
            for sc in range(1, SC):
                xtile = xtp.tile([128, EC, 512], BF16, tag="xt",
                                 name=f"xt_{sc}")
                nc.sync.dma_start(xtile[:], xt_d[sc])
                XSC.append(xtile)
            # Wo is first used ~40 iterations in; keep it behind all of x.
            nc.sync.dma_start(WO[:], wot_d)

            def xap(sc, ec):
                """x operand for contraction chunk ec of seq chunk sc."""
                if sc == 0:
                    return X0[ec // 2][:, ec % 2, :]
                return XSC[sc][:, ec, :]

            def wk_ap(ec, ft):
                return WKH[ec // 4][:, ec % 4, ft * 128:(ft + 1) * 128]

            def wq_ap(ec, ft):
                return WQH[ec // 4][:, ec % 4, ft * 128:(ft + 1) * 128]

            # ---- PE warmup during the input-DMA window ----
            # The HAM clock gate starts at half clock and needs ~3.4us of
            # sustained PE activity; burn cheap bf16 matmuls on junk data
            # while the inputs stream in so the first chains run warm.  They
            # write a scores-pool bank that is untouched until the first
            # scores, so they wait on nothing.  ~22 at the cold/warm mix ends
            # right as the first x sub-tiles land, so the first K/Q chains
            # pipeline behind the DMA instead of queueing behind junk.
            ps_w = scp.tile([128, 1024], F32, tag="sc", name="warmps")
            for _ in range(12):
                nc.tensor.matmul(ps_w[:, 0:512], warm[:, 0:128], warm[:, :],
                                 start=True, stop=True)

            # ---- filler unit scheduler ----
            # Each unit is a small PE/DVE/gpsimd burst; run_at is the
            # attention iteration (ic*32 + ft*16 + jt) it should run right
            # before.  run_at also fixes instruction emission order, so it
            # must respect dataflow.
            pending = []
            seq = [0]

            def add(run_at, fn):
                heapq.heappush(pending, (run_at, seq[0], fn))
                seq[0] += 1

            def pop_units(it):
                while pending and pending[0][0] <= it:
                    heapq.heappop(pending)[2]()

            # ---- projections ----
            # qT/kT: [f, s] layout.  out[f_tile, s_chunk] = sum_ec WqT^T @ xT
            QTs = {}
            KTs = {}

            def qk_proj_half(store, wap, ft, sc, half, state):
                """One half (4 ec chunks) of a q/k projection chain."""
                if half == 0:
                    state["ps"] = mmp.tile([128, 512], F32, tag="mm",
                                           name="qkps")
                ps = state["ps"]
                for ec in range(4 * half, 4 * half + 4):
                    nc.tensor.matmul(
                        ps[:, :],
                        wap(ec, ft),
                        xap(sc, ec),
                        start=(ec == 0), stop=(ec == EC - 1),
                    )
                if half == 1:
                    dst = qkp.tile([128, 512], BF16, tag="qk",
                                   name=f"qk_{ft}_{sc}_{len(store)}")
                    nc.vector.tensor_copy(dst[:], ps[:, :])
                    store[(ft, sc)] = dst

            def sched_qk(store, wap, ft, sc, run_at):
                st = {}
                add(run_at - 1,
                    lambda: qk_proj_half(store, wap, ft, sc, 0, st))
                add(run_at,
                    lambda: qk_proj_half(store, wap, ft, sc, 1, st))

            # v: natural [s, f] layout, plus a fused ones column per head:
            # VAUG[jt] is [128, HLOC, DK+1] with [:, h, DK] == 1.
            VAUG = [None] * ST

            def v_proj(jt):
                va = vp.tile([128, HLOC, DK + 1], BF16, tag="vaug")
                nc.vector.tensor_copy(va[:, :, DK:DK + 1],
                                      onescol[:, :, None])
                ps = mmp.tile([128, 512], F32, tag="mm", name="vps")
                sc = jt // 4
                for ec in range(EC):
                    nc.tensor.matmul(
                        ps[:, 0:FH],
                        xap(sc, ec)[:, (jt % 4) * 128:(jt % 4 + 1) * 128],
                        WV[:, ec, :],
                        start=(ec == 0), stop=(ec == EC - 1),
                    )
                nc.vector.tensor_copy(
                    va[:, :, 0:DK],
                    ps[:, 0:FH].rearrange("p (h d) -> p h d", d=DK))
                VAUG[jt] = va

            # ---- prologue: first K/Q chains, interleaved per-ec so both
            # pipeline behind the x0 sub-DMAs.  The Q drain goes through the
            # scalar engine (idle until the first exp) so the two drains
            # overlap instead of queueing on the DVE.
            ps_k = mmp.tile([128, 512], F32, tag="mm", name="kps0")
            ps_q = mmp.tile([128, 512], F32, tag="mm", name="qps0")
            for ec in range(EC):
                nc.tensor.matmul(ps_k[:, :], wk_ap(ec, 0), xap(0, ec),
                                 start=(ec == 0), stop=(ec == EC - 1))
                nc.tensor.matmul(ps_q[:, :], wq_ap(ec, 0), xap(0, ec),
                                 start=(ec == 0), stop=(ec == EC - 1))
            kt00 = qkp.tile([128, 512], BF16, tag="qk", name="kt00")
            nc.vector.tensor_copy(kt00[:], ps_k[:, :])
            KTs[(0, 0)] = kt00
            qt00 = qkp.tile([128, 512], BF16, tag="qk", name="qt00")
            nc.scalar.activation(qt00[:], ps_q[:, :],
                                 mybir.ActivationFunctionType.Copy)
            QTs[(0, 0)] = qt00

            # ---- filler schedule ----
            # V tiles just-in-time: VAUG[jt] is consumed by PV(jt), emitted at
            # iteration jt+1; popping at jt keeps each chain in its own slot.
            for _jt in range(ST):
                add(_jt, lambda jt=_jt: v_proj(jt))
            for _sc in range(1, SC):             # KTs[(0,sc)] used from 4*sc
                sched_qk(KTs, wk_ap, 0, _sc, 4 * _sc - 2)
            sched_qk(QTs, wq_ap, 1, 0, 11)       # used from iter 16
            for _sc in range(SC):                # KTs[(1,sc)] used from 16+4sc
                sched_qk(KTs, wk_ap, 1, _sc, 13 + 4 * _sc)
            for _ic in range(1, SC):
                # QTs[(0,ic)] is used from 32*ic; QTs[(1,ic)] from 32*ic+16.
                sched_qk(QTs, wq_ap, 0, _ic, 32 * _ic - 16)
                sched_qk(QTs, wq_ap, 1, _ic, 32 * _ic)

            def phase_c(ic, concat, start_at):
                # output projection: 8 tiles of [128 s, 512 e] per i chunk,
                # spread two iterations apart through the next chunk's loop.
                k = 0
                for stl in range(4):
                    st = ic * 4 + stl
                    for oc in range(2):
                        def emit(st=st, oc=oc, stl=stl, concat=concat):
                            ps_o = mmp.tile([128, 512], F32, tag="mm",
                                            name="ops")
                            for fc in range(FT):
                                nc.tensor.matmul(
                                    ps_o[:, :],
                                    concat[:, fc, stl * 128:(stl + 1) * 128],
                                    WO[:, fc, oc * 512:(oc + 1) * 512],
                                    start=(fc == 0), stop=(fc == FT - 1),
                                )
                            ot = op.tile([128, 512], BF16, tag="out")
                            if ic == SC - 1 and (stl + oc) % 2 == 0:
                                # tail tiles: the scalar engine is idle after
                                # the last exp, use it for half the drains
                                nc.scalar.activation(
                                    ot[:], ps_o[:, :],
                                    mybir.ActivationFunctionType.Copy)
                            else:
                                nc.vector.tensor_copy(ot[:], ps_o[:, :])
                            nc.sync.dma_start(
                                y_d[st * 128:(st + 1) * 128,
                                    oc * 512:(oc + 1) * 512],
                                ot[:])
                        add(start_at + 2 * k, emit)
                        k += 1

            def normalize_half(concat, aus, dnh, ft, start_at):
                # fast approx reciprocal (f32, ~18 bits) of this head pair's
                # denominators (rows 0/32 of dnh), cast to bf16 for the
                # broadcast matmuls, then two broadcast-matmul + multiply
                # pairs writing this pair's concat half.
                rdf = smp.tile([33, 512], F32, tag="rdf")
                rd = smp.tile([33, 512], BF16, tag="rd")

                def recip(rdf=rdf, rd=rd, dnh=dnh):
                    nc.vector.reciprocal_approx_fast(rdf[:], dnh[:])
                    nc.vector.tensor_copy(rd[:], rdf[:])
                add(start_at, recip)

                for hs in range(2):
                    # both heads' units at the same iteration: the broadcast
                    # matmuls then sit adjacent in the PE queue and overlap
                    # as concurrent 32-row groups
                    def norm_h(hs=hs, rd=rd, concat=concat, ft=ft,
                               aus=tuple(aus)):
                        pb = hs * DK
                        ps_b = mmp.tile([DK, 512], F32, tag="mm", name="bc")
                        nc.tensor.matmul(ps_b[:, :],
                                         ones[hs * 32:hs * 32 + 1, :],
                                         rd[hs * 32:hs * 32 + 1, :],
                                         start=True, stop=True,
                                         tile_position=(hs * 32, 0))
                        nc.vector.tensor_tensor(
                            concat[pb:pb + DK, ft, :], aus[hs][:, :],
                            ps_b[:, :], mybir.AluOpType.mult)
                    add(start_at + 1, norm_h)

            # ---- attention + output projection, per 512-wide i chunk ----
            # Heads are processed in pairs (partition bases 0/64); the K=64
            # score matmuls co-issue in distinct PE row groups.  The loop is
            # per j-tile: scores(jt), exp(jt), filler pop, then PV(jt-1), so
            # the exp stream stays back-to-back while the PE fills its slack
            # with projection/output units.
            for ic in range(SC):
                concat = cp.tile([128, FT, 512], BF16, tag="concat")
                base = ic * 32
                for ft in range(FT):
                    ps_h = [atp.tile([128, 512], F32, tag="at",
                                     name=f"at_{ic}_{ft}_{hs}")
                            for hs in range(2)]
                    prev_pv = None
                    for jt in range(ST):
                        ps_s = scp.tile([128, 1024], F32, tag="sc")
                        for hs in range(2):
                            pb = hs * DK
                            nc.tensor.matmul(
                                ps_s[:, hs * 512:(hs + 1) * 512],
                                KTs[(ft, jt // 4)][pb:pb + DK,
                                                   (jt % 4) * 128:
                                                   (jt % 4 + 1) * 128],
                                QTs[(ft, ic)][pb:pb + DK, :],
                                start=True, stop=True,
                            )
                        ex = ep.tile([128, 1024], mybir.dt.float16,
                                     tag="exp")
                        nc.scalar.activation(
                            ex[:], ps_s[:],
                            mybir.ActivationFunctionType.Exp,
                            bias=bias_t[:], scale=1.0 / np.sqrt(DK))
                        pop_units(base + ft * 16 + jt)
                        if prev_pv is not None:
                            prev_pv()

                        def pv(jt=jt, ft=ft, ps_h=ps_h, ex=ex):
                            for hs in range(2):
                                nc.tensor.matmul(
                                    ps_h[hs][0:DK + 1, :],
                                    VAUG[jt][:, ft * 2 + hs, :],
                                    ex[:, hs * 512:(hs + 1) * 512],
                                    start=(jt == 0), stop=(jt == ST - 1),
                                )
                        prev_pv = pv
                    prev_pv()
                    # free the attention psum quickly: attn rows via DVE
                    # copies; at the very end of the kernel (no exp follows)
                    # one attn copy and one denominator row go through the
                    # idle scalar engine as well.
                    last = (ic == SC - 1 and ft == FT - 1)
                    aus = []
                    dnh = smp.tile([33, 512], F32, tag="dn")
                    for hs in range(2):
                        au = aup.tile([DK, 512], BF16, tag="au")
                        if hs == 0 and last:
                            nc.scalar.activation(
                                au[:], ps_h[0][0:DK, :],
                                mybir.ActivationFunctionType.Copy)
                        else:
                            nc.vector.tensor_copy(au[:], ps_h[hs][0:DK, :])
                        aus.append(au)
                        if hs == 0 and last:
                            nc.scalar.activation(
                                dnh[0:1, :], ps_h[0][DK:DK + 1, :],
                                mybir.ActivationFunctionType.Copy)
                        else:
                            nc.vector.tensor_copy(
                                dnh[hs * 32:hs * 32 + 1, :],
                                ps_h[hs][DK:DK + 1, :])
                    normalize_half(concat, aus, dnh, ft,
                                   base + ft * 16 + 17)
                phase_c(ic, concat, base + 35)

            # Tail flush: the last chunk's normalize + output tiles.  The PE
            # would sit idle through the last pair's drain + reciprocal +
            # normalize chain (~3us) and the HAM clock gate would re-throttle
            # it to half rate, doubling every remaining output matmul; pad
            # just that window with junk warm matmuls (into a free scores
            # bank, so they wait on nothing), then run the output tiles
            # back-to-back at full clock.
            wps = scp.tile([128, 512], F32, tag="sc", name="tailwarm")
            for n, (_, _, fn) in enumerate(sorted(pending)):
                if n < 3:
                    for _ in range(4):
                        nc.tensor.matmul(wps[:, :], warm[:, 0:128],
                                         warm[:, :], start=True, stop=True)
                fn()

    nc.compile()
    return nc


_PROGRAM = None


def _get_program() -> bass.Bass:
    global _PROGRAM
    if _PROGRAM is None:
        _PROGRAM = _build_program()
    return _PROGRAM


def _prepare_in_maps(x, Wq, Wk, Wv, Wo):
    x = np.asarray(x, dtype=np.float32)
    Wq = np.asarray(Wq, dtype=np.float32)
    Wk = np.asarray(Wk, dtype=np.float32)
    Wv = np.asarray(Wv, dtype=np.float32)
    Wo = np.asarray(Wo, dtype=np.float32)
    bf = ml_dtypes.bfloat16
    in_maps = []
    for c in range(NCORES):
        b, hg = c // HGROUPS, c % HGROUPS
        rows = slice(hg * FH, (hg + 1) * FH)
        # device tile layouts: x -> [sc][p, c, s], W -> [p, c, f]
        # where the contraction index e = c*128 + p
        xt = x[b].T.reshape(E // 128, 128, S)
        xt = np.stack([xt[:, :, sc * 512:(sc + 1) * 512].transpose(1, 0, 2)
                       for sc in range(4)])
        wq = Wq[rows, :].T.reshape(E // 128, 128, FH).transpose(1, 0, 2)
        wk = Wk[rows, :].T.reshape(E // 128, 128, FH).transpose(1, 0, 2)
        wv = Wv[rows, :].T.reshape(E // 128, 128, FH).transpose(1, 0, 2)
        wo = Wo[:, rows].T.reshape(FH // 128, 128, E).transpose(1, 0, 2)
        in_maps.append({
            "xt": np.ascontiguousarray(xt).astype(bf),
            "wqt": np.ascontiguousarray(wq).astype(bf),
            "wkt": np.ascontiguousarray(wk).astype(bf),
            "wvt": np.ascontiguousarray(wv).astype(bf),
            "wot": np.ascontiguousarray(wo).astype(bf),
        })
    return in_maps


def run(inputs: dict, **spmd_kwargs):
    """Run on all 8 cores; returns (full output, BassKernelResults)."""
    nc = _get_program()
    in_maps = _prepare_in_maps(**inputs)
    res = run_bass_kernel_spmd(nc, in_maps, core_ids=list(range(NCORES)),
                               **spmd_kwargs)
    partials = [r["y"] for r in res.results]
    out = np.empty((B, S, E), dtype=np.float32)
    for b in range(B):
        acc = partials[b * HGROUPS].astype(np.float32, copy=True)
        for hg in range(1, HGROUPS):
            acc += partials[b * HGROUPS + hg]
        out[b] = acc
    return out, res


def kernel(**inputs) -> np.ndarray:
    out, _ = run(inputs)
    return out
